# revision 18
# baseline (speedup 1.0000x reference)
# Causal self-attention (B=2, T=2048, D=1024, H=16, HD=64) with RoPE on 8 TRN2
# cores — bf16 pipeline.
#
# Sharding: data-parallel over batch (2 groups of 4 cores), tensor-parallel
# over heads within each group (4 heads per core, as 2 pairs of 2). Each core:
#   - streams xT in bf16 while projecting q(pair0) and v(pair0) per k-tile,
#   - k(pair0) strip-major with per-strip RoPE so attention starts early,
#   - v is computed directly in [keys, hd] layout (no PE transposes),
#   - causal attention in S^T layout: exp on ACT, one static triangle mask
#     multiplied on DVE for diagonal blocks, ones-column in the AV lhsT
#     produces softmax denominators for free; AV lags S/exp by one block,
#   - pair-1 projections / v / RoPE are interleaved into pair-0's attention
#     (ACT-bound), out-proj strips are interleaved into pair-1's attention,
#   - q-strips processed in descending si so the tail strip is the smallest.
# The host sums the per-core partial [D, T] outputs and transposes back.
import sys
import os

sys.path.insert(0, "/opt/trn_rl_repo")

import numpy as np

import concourse.bass as bass  # noqa: F401
import concourse.mybir as mybir
from concourse import bacc
from concourse.tile import TileContext
from concourse.bass_utils import run_bass_kernel_spmd
from contextlib import ExitStack

F32 = mybir.dt.float32
BF16 = mybir.dt.bfloat16
AF = mybir.ActivationFunctionType
ALU = mybir.AluOpType

B, T, D = 2, 2048, 1024
H, HD = 16, 64
NCORES = 8
GROUPS = NCORES // B          # cores per batch = 4
HPC = H // GROUPS             # heads per core = 4
NK = D // 128                 # contraction tiles for D = 8
SCALE = HD ** -0.5

# hd interleave: new row 2j <- orig j, new row 2j+1 <- orig j+32 so the
# rotate-half partner of every row is its neighbour (swappable by a 32-lane
# stream shuffle).
PI = np.empty(HD, dtype=np.int64)
PI[0::2] = np.arange(32)
PI[1::2] = np.arange(32, 64)

SWAP_MASK = []
for _i in range(16):
    SWAP_MASK += [2 * _i + 1, 2 * _i]

# w image chunk order (each chunk is one [D, 128] column block, swizzled so
# SBUF partition rows are contiguous in dram)
WC_Q0, WC_K0, WC_V0, WC_V1, WC_Q1, WC_K1 = range(6)


def _build_program():
    nc = bacc.Bacc("TRN2", target_bir_lowering=False, debug=False,
                   num_devices=NCORES)
    d_xT = nc.dram_tensor("xT", [D, T], BF16, kind="ExternalInput").ap()
    d_w = nc.dram_tensor("wimg", [128, 6 * NK * 128], BF16,
                         kind="ExternalInput").ap()
    d_wo = nc.dram_tensor("woimg", [128, 2 * D], BF16,
                          kind="ExternalInput").ap()
    d_cos = nc.dram_tensor("cos2", [128, T], BF16, kind="ExternalInput").ap()
    d_sin = nc.dram_tensor("sin2", [128, T], BF16, kind="ExternalInput").ap()
    d_mask = nc.dram_tensor("trimask", [128, 256], BF16,
                            kind="ExternalInput").ap()
    d_out = nc.dram_tensor("outp", [D, T], BF16, kind="ExternalOutput").ap()
    dbg = bool(int(os.environ.get("KDEBUG", "0")))
    if dbg:
        d_dbg_q0 = nc.dram_tensor("dbg_q0", [128, T], BF16,
                                  kind="ExternalOutput").ap()
        d_dbg_k0 = nc.dram_tensor("dbg_k0", [128, T], BF16,
                                  kind="ExternalOutput").ap()
        d_dbg_va0 = nc.dram_tensor("dbg_va0", [128, 2 * 16 * 65], BF16,
                                   kind="ExternalOutput").ap()
        d_dbg_o0 = nc.dram_tensor("dbg_o0", [128, T], BF16,
                                  kind="ExternalOutput").ap()

    with TileContext(nc) as tc, nc.allow_low_precision(reason="bf16 attn"):
        with ExitStack() as root:
            persist = root.enter_context(tc.tile_pool(name="persist", bufs=1))

            qT = [persist.tile([128, T], BF16, tag=f"q{p}", name=f"qT{p}")
                  for p in range(2)]
            kT = [persist.tile([128, T], BF16, tag=f"k{p}", name=f"kT{p}")
                  for p in range(2)]
            vap = [persist.tile([128, 2 * 16 * 65], BF16, tag=f"va{p}",
                                name=f"vap{p}") for p in range(2)]
            oT = [persist.tile([128, T], BF16, tag=f"o{p}", name=f"oT{p}")
                  for p in range(2)]
            x_sb = [persist.tile([128, T], BF16, tag=f"x{kt}",
                                 name=f"xsb{kt}") for kt in range(NK)]
            w_sb = [persist.tile([128, NK * 128], BF16, tag=f"w{c}",
                                 name=f"wsb{c}") for c in range(6)]
            wo_sb = [persist.tile([128, D], BF16, tag=f"wo{p}",
                                  name=f"wo{p}") for p in range(2)]
            cos2 = persist.tile([128, T], BF16, tag="cos")
            sin2 = persist.tile([128, T], BF16, tag="sin")
            trimask = persist.tile([128, 256], BF16, tag="m")

            ropep = root.enter_context(tc.tile_pool(name="ropep", bufs=1))
            qc = [ropep.tile([128, T], BF16, tag=f"qc{i}", name=f"qc{i}")
                  for i in range(2)]
            qsh = [ropep.tile([128, T], BF16, tag=f"qsh{i}", name=f"qsh{i}")
                   for i in range(2)]
            qco = [ropep.tile([128, T], BF16, tag=f"qco{i}", name=f"qco{i}")
                   for i in range(2)]

            ptbp = root.enter_context(tc.tile_pool(name="ptbp", bufs=3))
            rp = root.enter_context(tc.tile_pool(name="rp", bufs=2))
            fop = root.enter_context(tc.tile_pool(name="fop", bufs=4))

            # [128, hl, kb, 65] views of vap
            vap3 = [vap[p][:].rearrange("a (h k c) -> a h k c", h=2, k=16)
                    for p in range(2)]
            tri3 = trimask[:].rearrange("a (h c) -> a h c", h=2)

            # ---------------- DMA preamble (sync queue, ordered) ----------
            def wslice(c):
                return d_w[:, c * NK * 128:(c + 1) * NK * 128]

            nc.sync.dma_start(out=w_sb[WC_Q0][:], in_=wslice(WC_Q0))
            nc.sync.dma_start(out=w_sb[WC_K0][:], in_=wslice(WC_K0))
            nc.sync.dma_start(out=x_sb[0][:, 0:1024], in_=d_xT[0:128, 0:1024])
            nc.sync.dma_start(out=x_sb[0][:, 1024:T],
                              in_=d_xT[0:128, 1024:T])
            for kt in range(1, NK):
                nc.sync.dma_start(out=x_sb[kt][:],
                                  in_=d_xT[kt * 128:(kt + 1) * 128, :])
            nc.sync.dma_start(out=w_sb[WC_V0][:], in_=wslice(WC_V0))
            nc.sync.dma_start(out=cos2[:], in_=d_cos[:])
            nc.sync.dma_start(out=sin2[:], in_=d_sin[:])
            nc.sync.dma_start(out=w_sb[WC_Q1][:], in_=wslice(WC_Q1))
            nc.sync.dma_start(out=w_sb[WC_K1][:], in_=wslice(WC_K1))
            nc.sync.dma_start(out=w_sb[WC_V1][:], in_=wslice(WC_V1))
            nc.sync.dma_start(out=wo_sb[0][:], in_=d_wo[:, 0:D])
            nc.sync.dma_start(out=wo_sb[1][:], in_=d_wo[:, D:2 * D])
            nc.sync.dma_start(out=trimask[:], in_=d_mask[:])

            for p in range(2):
                nc.vector.memset(vap3[p][:, :, :, 64:65], 1.0)

            # ---------------- rope helpers --------------------------------
            def rope_strip_ops(dst, buf, sl, src):
                """4 DVE ops turning src (bf16 or psum f32 [128, len(sl)])
                into roped dst[:, sl]."""
                nc.vector.tensor_tensor(out=qco[buf][:, sl], in0=src,
                                        in1=cos2[:, sl], op=ALU.mult)
                nc.vector.stream_shuffle(qsh[buf][:, sl], src, SWAP_MASK)
                nc.vector.tensor_tensor(out=qsh[buf][:, sl],
                                        in0=qsh[buf][:, sl],
                                        in1=sin2[:, sl], op=ALU.mult)
                nc.vector.tensor_tensor(out=dst[:, sl], in0=qsh[buf][:, sl],
                                        in1=qco[buf][:, sl], op=ALU.add)

            # ---------------- Phase A: x streaming + pair-0 q/k projs -----
            # q0 and k0 accumulate kt-major (one full psum bank per 512-col
            # strip — accumulation groups must own a whole bank: start=True
            # zeroes the full 2KB "zero region"). v is computed kb-major
            # later (bg generator) with one group per bank.
            psQ = tc.alloc_tile_pool(name="psQ", bufs=1, space="PSUM")
            psK = tc.alloc_tile_pool(name="psK", bufs=1, space="PSUM",
                                     side="right")
            pq = [psQ.tile([128, 512], F32, tag=f"pq{st}", name=f"pq{st}")
                  for st in range(4)]
            pk = [psK.tile([128, 512], F32, tag=f"pk{st}", name=f"pk{st}")
                  for st in range(4)]

            wq0 = w_sb[WC_Q0]
            wk0 = w_sb[WC_K0]
            for kt in range(NK):
                ks = slice(kt * 128, (kt + 1) * 128)
                for st in range(4):
                    nc.tensor.matmul(
                        pq[st][:], wq0[:, ks],
                        x_sb[kt][:, st * 512:(st + 1) * 512],
                        start=(kt == 0), stop=(kt == NK - 1))
                for st in range(4):
                    nc.tensor.matmul(
                        pk[st][:], wk0[:, ks],
                        x_sb[kt][:, st * 512:(st + 1) * 512],
                        start=(kt == 0), stop=(kt == NK - 1))

            # rope: q strips descending (si order), k strips ascending
            for qs, kst in ((3, 0), (2, 1), (1, 2), (0, 3)):
                slq = slice(qs * 512, (qs + 1) * 512)
                nc.scalar.copy(qc[0][:, slq], pq[qs][:])
                rope_strip_ops(qT[0], 0, slq, qc[0][:, slq])
                slk = slice(kst * 512, (kst + 1) * 512)
                nc.scalar.copy(qc[1][:, slk], pk[kst][:])
                rope_strip_ops(kT[0], 1, slk, qc[1][:, slk])
            psK.release()
            psQ.release()

            # ---------------- Phase B pools -------------------------------
            psS = tc.alloc_tile_pool(name="psS", bufs=2, space="PSUM",
                                     side="right")
            psA = tc.alloc_tile_pool(name="psA", bufs=1, space="PSUM")
            psBG = tc.alloc_tile_pool(name="psBG", bufs=2, space="PSUM")
            

            def bg_pair1():
                """v0 (kb-major), then q1/k1 proj+rope, then v1; one psum
                bank per accumulation group. Yields between PE chunks."""
                for p in range(2):
                    if p == 1:
                        # pair-1 q/k projections before its v
                        for wc, dstq, buf in ((WC_Q1, qT[1], 0),
                                              (WC_K1, kT[1], 1)):
                            w = w_sb[wc]
                            for st in range(4):
                                pj = psBG.tile([128, 512], F32, tag="bgk",
                                               name=f"bg{wc}_{st}")
                                for kt in range(0, NK, 2):
                                    for k2 in (kt, kt + 1):
                                        nc.tensor.matmul(
                                            pj[:],
                                            w[:, k2 * 128:(k2 + 1) * 128],
                                            x_sb[k2][:,
                                                     st * 512:(st + 1) * 512],
                                            start=(k2 == 0),
                                            stop=(k2 == NK - 1))
                                    yield
                                sl = slice(st * 512, (st + 1) * 512)
                                nc.vector.tensor_copy(qc[buf][:, sl], pj[:])
                                rope_strip_ops(dstq, buf, sl, qc[buf][:, sl])
                                yield
                    wv = w_sb[WC_V0 if p == 0 else WC_V1]
                    for kb in range(16):
                        pv = psBG.tile([128, 512], F32, tag="bgk",
                                       name=f"bgv{p}_{kb}")
                        for kt in range(NK):
                            nc.tensor.matmul(
                                pv[:, 0:128],
                                x_sb[kt][:, kb * 128:(kb + 1) * 128],
                                wv[:, kt * 128:(kt + 1) * 128],
                                start=(kt == 0), stop=(kt == NK - 1))
                        yield
                        if kb % 2 == 0:
                            nc.vector.tensor_copy(
                                vap3[p][:, :, kb, 0:64],
                                pv[:, 0:128].rearrange("a (h c) -> a h c",
                                                       h=2))
                        else:
                            nc.scalar.copy(
                                vap3[p][:, :, kb, 0:64],
                                pv[:, 0:128].rearrange("a (h c) -> a h c",
                                                       h=2))
                        yield

            def bg_oproj(si, pool, alt=False, t0=None, tw=512):
                if t0 is None:
                    t0 = si * 512
                for n in range(8):
                    pD = pool.tile([128, 512], F32, tag="pd",
                                   name=f"pD{si}_{n}_{t0}")
                    for p in range(2):
                        nc.tensor.matmul(
                            pD[:, 0:tw], wo_sb[p][:, n * 128:(n + 1) * 128],
                            oT[p][:, t0:t0 + tw],
                            start=(p == 0), stop=(p == 1))
                    fo = fop.tile([128, 512], BF16, tag="fo")
                    if alt and n % 2 == 1:
                        nc.scalar.copy(fo[:, 0:tw], pD[:, 0:tw])
                    else:
                        nc.vector.tensor_copy(fo[:, 0:tw], pD[:, 0:tw])
                    nc.sync.dma_start(
                        out=d_out[n * 128:(n + 1) * 128, t0:t0 + tw],
                        in_=fo[:, 0:tw])
                    yield

            def drain(gen, count=10 ** 9):
                if gen is None:
                    return
                for _ in range(count):
                    try:
                        next(gen)
                    except StopIteration:
                        return

            def attn_strip(si, p, bg=None, bg_per_kb=2, q_lo=None,
                           q_len=512):
                """Attention for q-strip si, pair p; AV lags one block."""
                q0 = 512 * si if q_lo is None else q_lo
                kb_max = (q0 + q_len) // 128
                av = [psA.tile([65, 512], F32, tag=f"av{hl}",
                               name=f"av{si}_{p}_{hl}_{q0}")
                      for hl in range(2)]

                def emit_av(st):
                    kb, ptb, o, L = st
                    for hl in range(2):
                        nc.tensor.matmul(
                            av[hl][:, o:q_len],
                            vap3[p][:, hl, kb, :],
                            ptb[:, 512 * hl:512 * hl + L],
                            start=(kb == 0), stop=(kb == kb_max - 1),
                            skip_group_check=True)

                prev = None
                for kb in range(kb_max):
                    o = max(0, 128 * kb - q0)
                    L = q_len - o
                    sps = psS.tile([128, 1024], F32, tag="sps",
                                   name=f"sps{si}_{p}_{kb}_{q0}")
                    for hl in range(2):
                        hb = 64 * hl
                        nc.tensor.matmul(
                            sps[:, 512 * hl + o:512 * hl + q_len],
                            kT[p][hb:hb + 64, kb * 128:(kb + 1) * 128],
                            qT[p][hb:hb + 64, q0 + o:q0 + q_len],
                            start=True, stop=True)
                    ptb = ptbp.tile([128, 1024], BF16, tag="ptb",
                                    name=f"ptb{si}_{p}_{kb}_{q0}")
                    sps3 = sps[:].rearrange("a (h q) -> a h q", h=2)
                    ptb3 = ptb[:].rearrange("a (h q) -> a h q", h=2)
                    nc.scalar.activation(ptb3[:, :, 0:L], sps3[:, :, o:q_len],
                                         AF.Exp, scale=SCALE)
                    if 128 * (kb + 1) > q0:
                        nc.vector.tensor_tensor(
                            out=ptb3[:, :, 0:128], in0=ptb3[:, :, 0:128],
                            in1=tri3, op=ALU.mult)
                    if prev is not None:
                        emit_av(prev)
                    prev = (kb, ptb, o, L)
                    if bg is not None:
                        drain(bg, count=bg_per_kb)
                emit_av(prev)
                # normalize -> oT strip
                for hl in range(2):
                    r_sb = rp.tile([1, 512], F32, tag=f"r{hl}",
                                   name=f"rsb{si}_{p}_{hl}_{q0}")
                    nc.vector.reciprocal(r_sb[:, 0:q_len], av[hl][64:65, 0:q_len])
                    rb = rp.tile([64, 512], F32, tag=f"rb{hl}",
                                 name=f"rbb{si}_{p}_{hl}_{q0}")
                    nc.gpsimd.partition_broadcast(rb[:, 0:q_len],
                                                  r_sb[:, 0:q_len])
                    nc.vector.tensor_tensor(
                        out=oT[p][64 * hl:64 * hl + 64, q0:q0 + q_len],
                        in0=av[hl][0:64, 0:q_len], in1=rb[:, 0:q_len],
                        op=ALU.mult)

            # ---------------- Phase B: attention --------------------------
            bg1 = bg_pair1()
            drain(bg1, count=6)
            for si in (3, 2, 1, 0):
                attn_strip(si, 0, bg=bg1, bg_per_kb=3 if si < 3 else 2)

            # leftover pair-1 work fills the first p1 strip (its out-proj
            # is not available yet)
            attn_strip(3, 1, bg=bg1, bg_per_kb=2)
            drain(bg1)
            psBG.release()
            psD = tc.alloc_tile_pool(name="psD", bufs=2, space="PSUM")
            obg = bg_oproj(3, psD)
            for si in (2, 1):
                attn_strip(si, 1, bg=obg, bg_per_kb=1)
                drain(obg)
                obg = bg_oproj(si, psD, alt=(si == 1))
            # last strip in two 256-col halves so its own out-proj overlaps
            attn_strip(0, 1, bg=obg, bg_per_kb=1, q_lo=0, q_len=256)
            drain(obg)
            obg = bg_oproj(0, psD, alt=True, t0=0, tw=256)
            attn_strip(0, 1, bg=obg, bg_per_kb=2, q_lo=256, q_len=256)
            drain(obg)
            obg = None

            psD.release()
            psA.release()
            psS.release()

            # tail: out-projection of the last (smallest) strip with all
            # banks available
            psD2 = tc.alloc_tile_pool(name="psD2", bufs=4, space="PSUM")
            drain(bg_oproj(0, psD2, alt=True, t0=256, tw=256))
            psD2.release()

            if dbg:
                nc.sync.dma_start(out=d_dbg_q0[:], in_=qT[0][:])
                nc.sync.dma_start(out=d_dbg_k0[:], in_=kT[0][:])
                nc.sync.dma_start(out=d_dbg_va0[:], in_=vap[0][:])
                nc.sync.dma_start(out=d_dbg_o0[:], in_=oT[0][:])

    nc.compile()
    return nc


_NC_CACHE = None


def _get_program():
    global _NC_CACHE
    if _NC_CACHE is None:
        _NC_CACHE = _build_program()
    return _NC_CACHE


def _rope_tables():
    inv_freq = 1.0 / (10000.0 ** (np.arange(0, HD, 2, dtype=np.float32) / HD))
    freqs = np.outer(np.arange(T, dtype=np.float32), inv_freq)  # [T, 32]
    emb = np.concatenate([freqs, freqs], axis=-1)               # [T, 64]
    return np.cos(emb), np.sin(emb)


def _to_bf16(a):
    import ml_dtypes
    return np.asarray(a, dtype=np.float32).astype(ml_dtypes.bfloat16)


def _swizzle_w(wcol):
    """[D, 128] column block -> SBUF image [128, NK*128] with
    img[p, kt*128 + j] = wcol[kt*128 + p, j]."""
    w3 = wcol.reshape(NK, 128, 128)          # [kt, p, j]
    return np.ascontiguousarray(w3.transpose(1, 0, 2).reshape(128, NK * 128))


def _host_prep(x, w_qkv, w_out):
    cos, sin = _rope_tables()
    cosP = np.ascontiguousarray(cos.T[PI, :])                   # [64, T]
    sinP = sin.T[PI, :].copy()
    sinP[0::2, :] *= -1.0                                       # sign baked in
    cos2 = _to_bf16(np.vstack([cosP, cosP]))
    sin2 = _to_bf16(np.vstack([sinP, sinP]))
    tri = np.triu(np.ones((128, 128), dtype=np.float32))        # keep j >= i
    trimask = _to_bf16(np.ascontiguousarray(
        np.concatenate([tri, tri], axis=1)))                    # [128, 256]

    in_maps = []
    for core in range(NCORES):
        b = core // GROUPS
        h0 = (core % GROUPS) * HPC
        xT = _to_bf16(np.ascontiguousarray(x[b].T))             # [D, T]

        def wcolq(kind, pair):                                  # permuted
            cols = []
            for hh in range(2):
                h = h0 + 2 * pair + hh
                wcol = w_qkv[:, kind * D + h * HD:kind * D + (h + 1) * HD]
                cols.append(wcol[:, PI])
            return np.concatenate(cols, axis=1)                 # [D, 128]

        def wcolv(pair):
            cols = []
            for hh in range(2):
                h = h0 + 2 * pair + hh
                cols.append(w_qkv[:, 2 * D + h * HD:2 * D + (h + 1) * HD])
            return np.concatenate(cols, axis=1)

        order = [wcolq(0, 0), wcolq(1, 0), wcolv(0), wcolv(1),
                 wcolq(0, 1), wcolq(1, 1)]
        wimg = np.concatenate([_swizzle_w(c) for c in order], axis=1)
        wimg = np.ascontiguousarray(_to_bf16(wimg))             # [128, 6144]

        wo_rows = w_out[h0 * HD:(h0 + HPC) * HD, :]             # [256, D]
        woimg = np.ascontiguousarray(_to_bf16(
            np.concatenate([wo_rows[0:128, :], wo_rows[128:256, :]],
                           axis=1)))                            # [128, 2D]

        in_maps.append({
            "xT": xT,
            "wimg": wimg,
            "woimg": woimg,
            "cos2": cos2,
            "sin2": sin2,
            "trimask": trimask,
        })
    return in_maps


def kernel(x, w_qkv, w_out):
    x = np.asarray(x, dtype=np.float32)
    w_qkv = np.asarray(w_qkv, dtype=np.float32)
    w_out = np.asarray(w_out, dtype=np.float32)
    nc = _get_program()
    in_maps = _host_prep(x, w_qkv, w_out)
    trace = bool(int(os.environ.get("KBENCH_TRACE", "0")))
    res = run_bass_kernel_spmd(nc, in_maps, list(range(NCORES)), trace=trace)
    if trace and res.exec_time_ns is not None:
        print(f"HW exec time: {res.exec_time_ns} ns")
    out = np.zeros((B, T, D), dtype=np.float32)
    for core in range(NCORES):
        b = core // GROUPS
        out[b] += res.results[core]["outp"].T.astype(np.float32)
    return out


# revision 19
# speedup vs baseline: 1.0084x; 1.0084x over previous
# Causal self-attention (B=2, T=2048, D=1024, H=16, HD=64) with RoPE on 8 TRN2
# cores — bf16 pipeline.
#
# Sharding: data-parallel over batch (2 groups of 4 cores), tensor-parallel
# over heads within each group (4 heads per core, as 2 pairs of 2). Each core:
#   - streams xT in bf16 while projecting q(pair0) and v(pair0) per k-tile,
#   - k(pair0) strip-major with per-strip RoPE so attention starts early,
#   - v is computed directly in [keys, hd] layout (no PE transposes),
#   - causal attention in S^T layout: exp on ACT, one static triangle mask
#     multiplied on DVE for diagonal blocks, ones-column in the AV lhsT
#     produces softmax denominators for free; AV lags S/exp by one block,
#   - pair-1 projections / v / RoPE are interleaved into pair-0's attention
#     (ACT-bound), out-proj strips are interleaved into pair-1's attention,
#   - q-strips processed in descending si so the tail strip is the smallest.
# The host sums the per-core partial [D, T] outputs and transposes back.
import sys
import os

sys.path.insert(0, "/opt/trn_rl_repo")

import numpy as np

import concourse.bass as bass  # noqa: F401
import concourse.mybir as mybir
from concourse import bacc
from concourse.tile import TileContext
from concourse.bass_utils import run_bass_kernel_spmd
from contextlib import ExitStack

F32 = mybir.dt.float32
BF16 = mybir.dt.bfloat16
AF = mybir.ActivationFunctionType
ALU = mybir.AluOpType

B, T, D = 2, 2048, 1024
H, HD = 16, 64
NCORES = 8
GROUPS = NCORES // B          # cores per batch = 4
HPC = H // GROUPS             # heads per core = 4
NK = D // 128                 # contraction tiles for D = 8
SCALE = HD ** -0.5

# hd interleave: new row 2j <- orig j, new row 2j+1 <- orig j+32 so the
# rotate-half partner of every row is its neighbour (swappable by a 32-lane
# stream shuffle).
PI = np.empty(HD, dtype=np.int64)
PI[0::2] = np.arange(32)
PI[1::2] = np.arange(32, 64)

SWAP_MASK = []
for _i in range(16):
    SWAP_MASK += [2 * _i + 1, 2 * _i]

# w image chunk order (each chunk is one [D, 128] column block, swizzled so
# SBUF partition rows are contiguous in dram)
WC_Q0, WC_K0, WC_V0, WC_V1, WC_Q1, WC_K1 = range(6)


def _build_program():
    nc = bacc.Bacc("TRN2", target_bir_lowering=False, debug=False,
                   num_devices=NCORES)
    d_xT = nc.dram_tensor("xT", [D, T], BF16, kind="ExternalInput").ap()
    d_w = nc.dram_tensor("wimg", [128, 6 * NK * 128], BF16,
                         kind="ExternalInput").ap()
    d_wo = nc.dram_tensor("woimg", [128, 2 * D], BF16,
                          kind="ExternalInput").ap()
    d_cos = nc.dram_tensor("cos2", [128, T], BF16, kind="ExternalInput").ap()
    d_sin = nc.dram_tensor("sin2", [128, T], BF16, kind="ExternalInput").ap()
    d_mask = nc.dram_tensor("trimask", [128, 256], BF16,
                            kind="ExternalInput").ap()
    d_out = nc.dram_tensor("outp", [D, T], BF16, kind="ExternalOutput").ap()
    dbg = bool(int(os.environ.get("KDEBUG", "0")))
    if dbg:
        d_dbg_q0 = nc.dram_tensor("dbg_q0", [128, T], BF16,
                                  kind="ExternalOutput").ap()
        d_dbg_k0 = nc.dram_tensor("dbg_k0", [128, T], BF16,
                                  kind="ExternalOutput").ap()
        d_dbg_va0 = nc.dram_tensor("dbg_va0", [128, 2 * 16 * 65], BF16,
                                   kind="ExternalOutput").ap()
        d_dbg_o0 = nc.dram_tensor("dbg_o0", [128, T], BF16,
                                  kind="ExternalOutput").ap()

    with TileContext(nc) as tc, nc.allow_low_precision(reason="bf16 attn"):
        with ExitStack() as root:
            persist = root.enter_context(tc.tile_pool(name="persist", bufs=1))

            qT = [persist.tile([128, T], BF16, tag=f"q{p}", name=f"qT{p}")
                  for p in range(2)]
            kT = [persist.tile([128, T], BF16, tag=f"k{p}", name=f"kT{p}")
                  for p in range(2)]
            vap = [persist.tile([128, 2 * 16 * 65], BF16, tag=f"va{p}",
                                name=f"vap{p}") for p in range(2)]
            oT = [persist.tile([128, T], BF16, tag=f"o{p}", name=f"oT{p}")
                  for p in range(2)]
            x_sb = [persist.tile([128, T], BF16, tag=f"x{kt}",
                                 name=f"xsb{kt}") for kt in range(NK)]
            w_sb = [persist.tile([128, NK * 128], BF16, tag=f"w{c}",
                                 name=f"wsb{c}") for c in range(6)]
            wo_sb = [persist.tile([128, D], BF16, tag=f"wo{p}",
                                  name=f"wo{p}") for p in range(2)]
            cos2 = persist.tile([128, T], BF16, tag="cos")
            sin2 = persist.tile([128, T], BF16, tag="sin")
            trimask = persist.tile([128, 256], BF16, tag="m")

            ropep = root.enter_context(tc.tile_pool(name="ropep", bufs=1))
            qc = [ropep.tile([128, T], BF16, tag=f"qc{i}", name=f"qc{i}")
                  for i in range(2)]
            qsh = [ropep.tile([128, T], BF16, tag=f"qsh{i}", name=f"qsh{i}")
                   for i in range(2)]
            qco = [ropep.tile([128, T], BF16, tag=f"qco{i}", name=f"qco{i}")
                   for i in range(2)]

            ptbp = root.enter_context(tc.tile_pool(name="ptbp", bufs=3))
            rp = root.enter_context(tc.tile_pool(name="rp", bufs=2))
            fop = root.enter_context(tc.tile_pool(name="fop", bufs=8))

            # [128, hl, kb, 65] views of vap
            vap3 = [vap[p][:].rearrange("a (h k c) -> a h k c", h=2, k=16)
                    for p in range(2)]
            tri3 = trimask[:].rearrange("a (h c) -> a h c", h=2)

            # ---------------- DMA preamble (sync queue, ordered) ----------
            def wslice(c):
                return d_w[:, c * NK * 128:(c + 1) * NK * 128]

            nc.sync.dma_start(out=w_sb[WC_Q0][:], in_=wslice(WC_Q0))
            nc.sync.dma_start(out=w_sb[WC_K0][:], in_=wslice(WC_K0))
            for kt in range(0, NK):
                nc.sync.dma_start(out=x_sb[kt][:],
                                  in_=d_xT[kt * 128:(kt + 1) * 128, :])
            nc.sync.dma_start(out=w_sb[WC_V0][:], in_=wslice(WC_V0))
            nc.sync.dma_start(out=cos2[:], in_=d_cos[:])
            nc.sync.dma_start(out=sin2[:], in_=d_sin[:])
            nc.sync.dma_start(out=w_sb[WC_Q1][:], in_=wslice(WC_Q1))
            nc.sync.dma_start(out=w_sb[WC_K1][:], in_=wslice(WC_K1))
            nc.sync.dma_start(out=w_sb[WC_V1][:], in_=wslice(WC_V1))
            nc.sync.dma_start(out=wo_sb[0][:], in_=d_wo[:, 0:D])
            nc.sync.dma_start(out=wo_sb[1][:], in_=d_wo[:, D:2 * D])
            nc.sync.dma_start(out=trimask[:], in_=d_mask[:])

            for p in range(2):
                nc.vector.memset(vap3[p][:, :, :, 64:65], 1.0)

            # ---------------- rope helpers --------------------------------
            def rope_strip_ops(dst, buf, sl, src):
                """4 DVE ops turning src (bf16 or psum f32 [128, len(sl)])
                into roped dst[:, sl]."""
                nc.vector.tensor_tensor(out=qco[buf][:, sl], in0=src,
                                        in1=cos2[:, sl], op=ALU.mult)
                nc.vector.stream_shuffle(qsh[buf][:, sl], src, SWAP_MASK)
                nc.vector.tensor_tensor(out=qsh[buf][:, sl],
                                        in0=qsh[buf][:, sl],
                                        in1=sin2[:, sl], op=ALU.mult)
                nc.vector.tensor_tensor(out=dst[:, sl], in0=qsh[buf][:, sl],
                                        in1=qco[buf][:, sl], op=ALU.add)

            # ---------------- Phase A: x streaming + pair-0 q/k projs -----
            # q0 and k0 accumulate kt-major (one full psum bank per 512-col
            # strip — accumulation groups must own a whole bank: start=True
            # zeroes the full 2KB "zero region"). v is computed kb-major
            # later (bg generator) with one group per bank.
            psQ = tc.alloc_tile_pool(name="psQ", bufs=1, space="PSUM")
            psK = tc.alloc_tile_pool(name="psK", bufs=1, space="PSUM",
                                     side="right")
            pq = [psQ.tile([128, 512], F32, tag=f"pq{st}", name=f"pq{st}")
                  for st in range(4)]
            pk = [psK.tile([128, 512], F32, tag=f"pk{st}", name=f"pk{st}")
                  for st in range(4)]

            wq0 = w_sb[WC_Q0]
            wk0 = w_sb[WC_K0]
            for kt in range(NK):
                ks = slice(kt * 128, (kt + 1) * 128)
                for st in range(4):
                    nc.tensor.matmul(
                        pq[st][:], wq0[:, ks],
                        x_sb[kt][:, st * 512:(st + 1) * 512],
                        start=(kt == 0), stop=(kt == NK - 1))
                for st in range(4):
                    nc.tensor.matmul(
                        pk[st][:], wk0[:, ks],
                        x_sb[kt][:, st * 512:(st + 1) * 512],
                        start=(kt == 0), stop=(kt == NK - 1))

            # rope: q strips descending (si order), k strips ascending
            for qs, kst in ((3, 0), (2, 1), (1, 2), (0, 3)):
                slq = slice(qs * 512, (qs + 1) * 512)
                nc.scalar.copy(qc[0][:, slq], pq[qs][:])
                rope_strip_ops(qT[0], 0, slq, qc[0][:, slq])
                slk = slice(kst * 512, (kst + 1) * 512)
                nc.scalar.copy(qc[1][:, slk], pk[kst][:])
                rope_strip_ops(kT[0], 1, slk, qc[1][:, slk])
            psK.release()
            psQ.release()

            # ---------------- Phase B pools -------------------------------
            psS = tc.alloc_tile_pool(name="psS", bufs=2, space="PSUM",
                                     side="right")
            psA = tc.alloc_tile_pool(name="psA", bufs=1, space="PSUM")
            psBG = tc.alloc_tile_pool(name="psBG", bufs=2, space="PSUM")
            

            def bg_pair1():
                """v0 (kb-major), then q1/k1 proj+rope, then v1; one psum
                bank per accumulation group. Yields between PE chunks."""
                for p in range(2):
                    if p == 1:
                        # pair-1 q/k projections before its v
                        for wc, dstq, buf in ((WC_Q1, qT[1], 0),
                                              (WC_K1, kT[1], 1)):
                            w = w_sb[wc]
                            for st in range(4):
                                pj = psBG.tile([128, 512], F32, tag="bgk",
                                               name=f"bg{wc}_{st}")
                                for kt in range(0, NK, 2):
                                    for k2 in (kt, kt + 1):
                                        nc.tensor.matmul(
                                            pj[:],
                                            w[:, k2 * 128:(k2 + 1) * 128],
                                            x_sb[k2][:,
                                                     st * 512:(st + 1) * 512],
                                            start=(k2 == 0),
                                            stop=(k2 == NK - 1))
                                    yield
                                sl = slice(st * 512, (st + 1) * 512)
                                nc.vector.tensor_copy(qc[buf][:, sl], pj[:])
                                rope_strip_ops(dstq, buf, sl, qc[buf][:, sl])
                                yield
                    wv = w_sb[WC_V0 if p == 0 else WC_V1]
                    for kb in range(16):
                        pv = psBG.tile([128, 512], F32, tag="bgk",
                                       name=f"bgv{p}_{kb}")
                        for kt in range(NK):
                            nc.tensor.matmul(
                                pv[:, 0:128],
                                x_sb[kt][:, kb * 128:(kb + 1) * 128],
                                wv[:, kt * 128:(kt + 1) * 128],
                                start=(kt == 0), stop=(kt == NK - 1))
                        yield
                        if kb % 2 == 0:
                            nc.vector.tensor_copy(
                                vap3[p][:, :, kb, 0:64],
                                pv[:, 0:128].rearrange("a (h c) -> a h c",
                                                       h=2))
                        else:
                            nc.scalar.copy(
                                vap3[p][:, :, kb, 0:64],
                                pv[:, 0:128].rearrange("a (h c) -> a h c",
                                                       h=2))
                        yield

            def bg_oproj(si, pool, alt=False, t0=None, tw=512):
                if t0 is None:
                    t0 = si * 512
                for n in range(8):
                    pD = pool.tile([128, 512], F32, tag="pd",
                                   name=f"pD{si}_{n}_{t0}")
                    for p in range(2):
                        nc.tensor.matmul(
                            pD[:, 0:tw], wo_sb[p][:, n * 128:(n + 1) * 128],
                            oT[p][:, t0:t0 + tw],
                            start=(p == 0), stop=(p == 1))
                    fo = fop.tile([128, 512], BF16, tag="fo")
                    if alt and n % 2 == 1:
                        nc.scalar.copy(fo[:, 0:tw], pD[:, 0:tw])
                    else:
                        nc.vector.tensor_copy(fo[:, 0:tw], pD[:, 0:tw])
                    nc.sync.dma_start(
                        out=d_out[n * 128:(n + 1) * 128, t0:t0 + tw],
                        in_=fo[:, 0:tw])
                    yield

            def drain(gen, count=10 ** 9):
                if gen is None:
                    return
                for _ in range(count):
                    try:
                        next(gen)
                    except StopIteration:
                        return

            def attn_strip(si, p, bg=None, bg_per_kb=2, q_lo=None,
                           q_len=512):
                """Attention for q-strip si, pair p; AV lags one block."""
                q0 = 512 * si if q_lo is None else q_lo
                kb_max = (q0 + q_len) // 128
                av = [psA.tile([65, 512], F32, tag=f"av{hl}",
                               name=f"av{si}_{p}_{hl}_{q0}")
                      for hl in range(2)]

                def emit_av(st):
                    kb, ptb, o, L = st
                    for hl in range(2):
                        nc.tensor.matmul(
                            av[hl][:, o:q_len],
                            vap3[p][:, hl, kb, :],
                            ptb[:, 512 * hl:512 * hl + L],
                            start=(kb == 0), stop=(kb == kb_max - 1),
                            skip_group_check=True)

                prev = None
                for kb in range(kb_max):
                    o = max(0, 128 * kb - q0)
                    L = q_len - o
                    sps = psS.tile([128, 1024], F32, tag="sps",
                                   name=f"sps{si}_{p}_{kb}_{q0}")
                    for hl in range(2):
                        hb = 64 * hl
                        nc.tensor.matmul(
                            sps[:, 512 * hl + o:512 * hl + q_len],
                            kT[p][hb:hb + 64, kb * 128:(kb + 1) * 128],
                            qT[p][hb:hb + 64, q0 + o:q0 + q_len],
                            start=True, stop=True)
                    ptb = ptbp.tile([128, 1024], BF16, tag="ptb",
                                    name=f"ptb{si}_{p}_{kb}_{q0}")
                    sps3 = sps[:].rearrange("a (h q) -> a h q", h=2)
                    ptb3 = ptb[:].rearrange("a (h q) -> a h q", h=2)
                    nc.scalar.activation(ptb3[:, :, 0:L], sps3[:, :, o:q_len],
                                         AF.Exp, scale=SCALE)
                    if 128 * (kb + 1) > q0:
                        nc.vector.tensor_tensor(
                            out=ptb3[:, :, 0:128], in0=ptb3[:, :, 0:128],
                            in1=tri3, op=ALU.mult)
                    if prev is not None:
                        emit_av(prev)
                    prev = (kb, ptb, o, L)
                    if bg is not None:
                        drain(bg, count=bg_per_kb)
                emit_av(prev)
                # normalize -> oT strip
                for hl in range(2):
                    r_sb = rp.tile([1, 512], F32, tag=f"r{hl}",
                                   name=f"rsb{si}_{p}_{hl}_{q0}")
                    nc.vector.reciprocal(r_sb[:, 0:q_len], av[hl][64:65, 0:q_len])
                    rb = rp.tile([64, 512], F32, tag=f"rb{hl}",
                                 name=f"rbb{si}_{p}_{hl}_{q0}")
                    nc.gpsimd.partition_broadcast(rb[:, 0:q_len],
                                                  r_sb[:, 0:q_len])
                    nc.vector.tensor_tensor(
                        out=oT[p][64 * hl:64 * hl + 64, q0:q0 + q_len],
                        in0=av[hl][0:64, 0:q_len], in1=rb[:, 0:q_len],
                        op=ALU.mult)

            # ---------------- Phase B: attention --------------------------
            bg1 = bg_pair1()
            drain(bg1, count=6)
            for si in (3, 2, 1, 0):
                attn_strip(si, 0, bg=bg1, bg_per_kb=3 if si < 3 else 2)

            # leftover pair-1 work fills the first p1 strip (its out-proj
            # is not available yet)
            attn_strip(3, 1, bg=bg1, bg_per_kb=2)
            drain(bg1)
            psBG.release()
            psD = tc.alloc_tile_pool(name="psD", bufs=2, space="PSUM")
            obg = bg_oproj(3, psD)
            for si in (2, 1):
                attn_strip(si, 1, bg=obg, bg_per_kb=1)
                drain(obg)
                obg = bg_oproj(si, psD, alt=(si == 1))
            # last strip in two 256-col halves so its own out-proj overlaps
            attn_strip(0, 1, bg=obg, bg_per_kb=1, q_lo=0, q_len=256)
            drain(obg)
            obg = bg_oproj(0, psD, alt=True, t0=0, tw=256)
            attn_strip(0, 1, bg=obg, bg_per_kb=2, q_lo=256, q_len=256)
            drain(obg)
            obg = None

            psD.release()
            psA.release()
            psS.release()

            # tail: out-projection of the last (smallest) strip with all
            # banks available
            psD2 = tc.alloc_tile_pool(name="psD2", bufs=4, space="PSUM")
            drain(bg_oproj(0, psD2, alt=True, t0=256, tw=256))
            psD2.release()

            if dbg:
                nc.sync.dma_start(out=d_dbg_q0[:], in_=qT[0][:])
                nc.sync.dma_start(out=d_dbg_k0[:], in_=kT[0][:])
                nc.sync.dma_start(out=d_dbg_va0[:], in_=vap[0][:])
                nc.sync.dma_start(out=d_dbg_o0[:], in_=oT[0][:])

    nc.compile()
    return nc


_NC_CACHE = None


def _get_program():
    global _NC_CACHE
    if _NC_CACHE is None:
        _NC_CACHE = _build_program()
    return _NC_CACHE


def _rope_tables():
    inv_freq = 1.0 / (10000.0 ** (np.arange(0, HD, 2, dtype=np.float32) / HD))
    freqs = np.outer(np.arange(T, dtype=np.float32), inv_freq)  # [T, 32]
    emb = np.concatenate([freqs, freqs], axis=-1)               # [T, 64]
    return np.cos(emb), np.sin(emb)


def _to_bf16(a):
    import ml_dtypes
    return np.asarray(a, dtype=np.float32).astype(ml_dtypes.bfloat16)


def _swizzle_w(wcol):
    """[D, 128] column block -> SBUF image [128, NK*128] with
    img[p, kt*128 + j] = wcol[kt*128 + p, j]."""
    w3 = wcol.reshape(NK, 128, 128)          # [kt, p, j]
    return np.ascontiguousarray(w3.transpose(1, 0, 2).reshape(128, NK * 128))


def _host_prep(x, w_qkv, w_out):
    cos, sin = _rope_tables()
    cosP = np.ascontiguousarray(cos.T[PI, :])                   # [64, T]
    sinP = sin.T[PI, :].copy()
    sinP[0::2, :] *= -1.0                                       # sign baked in
    cos2 = _to_bf16(np.vstack([cosP, cosP]))
    sin2 = _to_bf16(np.vstack([sinP, sinP]))
    tri = np.triu(np.ones((128, 128), dtype=np.float32))        # keep j >= i
    trimask = _to_bf16(np.ascontiguousarray(
        np.concatenate([tri, tri], axis=1)))                    # [128, 256]

    in_maps = []
    for core in range(NCORES):
        b = core // GROUPS
        h0 = (core % GROUPS) * HPC
        xT = _to_bf16(np.ascontiguousarray(x[b].T))             # [D, T]

        def wcolq(kind, pair):                                  # permuted
            cols = []
            for hh in range(2):
                h = h0 + 2 * pair + hh
                wcol = w_qkv[:, kind * D + h * HD:kind * D + (h + 1) * HD]
                cols.append(wcol[:, PI])
            return np.concatenate(cols, axis=1)                 # [D, 128]

        def wcolv(pair):
            cols = []
            for hh in range(2):
                h = h0 + 2 * pair + hh
                cols.append(w_qkv[:, 2 * D + h * HD:2 * D + (h + 1) * HD])
            return np.concatenate(cols, axis=1)

        order = [wcolq(0, 0), wcolq(1, 0), wcolv(0), wcolv(1),
                 wcolq(0, 1), wcolq(1, 1)]
        wimg = np.concatenate([_swizzle_w(c) for c in order], axis=1)
        wimg = np.ascontiguousarray(_to_bf16(wimg))             # [128, 6144]

        wo_rows = w_out[h0 * HD:(h0 + HPC) * HD, :]             # [256, D]
        woimg = np.ascontiguousarray(_to_bf16(
            np.concatenate([wo_rows[0:128, :], wo_rows[128:256, :]],
                           axis=1)))                            # [128, 2D]

        in_maps.append({
            "xT": xT,
            "wimg": wimg,
            "woimg": woimg,
            "cos2": cos2,
            "sin2": sin2,
            "trimask": trimask,
        })
    return in_maps


def kernel(x, w_qkv, w_out):
    x = np.asarray(x, dtype=np.float32)
    w_qkv = np.asarray(w_qkv, dtype=np.float32)
    w_out = np.asarray(w_out, dtype=np.float32)
    nc = _get_program()
    in_maps = _host_prep(x, w_qkv, w_out)
    trace = bool(int(os.environ.get("KBENCH_TRACE", "0")))
    res = run_bass_kernel_spmd(nc, in_maps, list(range(NCORES)), trace=trace)
    if trace and res.exec_time_ns is not None:
        print(f"HW exec time: {res.exec_time_ns} ns")
    out = np.zeros((B, T, D), dtype=np.float32)
    for core in range(NCORES):
        b = core // GROUPS
        out[b] += res.results[core]["outp"].T.astype(np.float32)
    return out


# revision 22
# speedup vs baseline: 1.0880x; 1.0790x over previous
# Causal self-attention (B=2, T=2048, D=1024, H=16, HD=64) with RoPE on 8 TRN2
# cores — bf16 pipeline.
#
# Sharding: data-parallel over batch (2 groups of 4 cores), tensor-parallel
# over heads within each group (4 heads per core, as 2 pairs of 2). Each core:
#   - streams xT in bf16 while projecting q(pair0) and v(pair0) per k-tile,
#   - k(pair0) strip-major with per-strip RoPE so attention starts early,
#   - v is computed directly in [keys, hd] layout (no PE transposes),
#   - causal attention in S^T layout: exp on ACT, one static triangle mask
#     multiplied on DVE for diagonal blocks, ones-column in the AV lhsT
#     produces softmax denominators for free; AV lags S/exp by one block,
#   - pair-1 projections / v / RoPE are interleaved into pair-0's attention
#     (ACT-bound), out-proj strips are interleaved into pair-1's attention,
#   - q-strips processed in descending si so the tail strip is the smallest.
# The host sums the per-core partial [D, T] outputs and transposes back.
import sys
import os

sys.path.insert(0, "/opt/trn_rl_repo")

import numpy as np

import concourse.bass as bass  # noqa: F401
import concourse.mybir as mybir
from concourse import bacc
from concourse.tile import TileContext
from concourse.bass_utils import run_bass_kernel_spmd
from contextlib import ExitStack

F32 = mybir.dt.float32
BF16 = mybir.dt.bfloat16
AF = mybir.ActivationFunctionType
ALU = mybir.AluOpType

B, T, D = 2, 2048, 1024
H, HD = 16, 64
NCORES = 8
GROUPS = NCORES // B          # cores per batch = 4
HPC = H // GROUPS             # heads per core = 4
NK = D // 128                 # contraction tiles for D = 8
SCALE = HD ** -0.5

# hd interleave: new row 2j <- orig j, new row 2j+1 <- orig j+32 so the
# rotate-half partner of every row is its neighbour (swappable by a 32-lane
# stream shuffle).
PI = np.empty(HD, dtype=np.int64)
PI[0::2] = np.arange(32)
PI[1::2] = np.arange(32, 64)

SWAP_MASK = []
for _i in range(16):
    SWAP_MASK += [2 * _i + 1, 2 * _i]

# w image chunk order (each chunk is one [D, 128] column block, swizzled so
# SBUF partition rows are contiguous in dram)
WC_Q0, WC_K0, WC_V0, WC_V1, WC_Q1, WC_K1 = range(6)


def _build_program():
    nc = bacc.Bacc("TRN2", target_bir_lowering=False, debug=False,
                   num_devices=NCORES)
    d_xT = nc.dram_tensor("xT", [D, T], BF16, kind="ExternalInput").ap()
    d_w = nc.dram_tensor("wimg", [128, 6 * NK * 128], BF16,
                         kind="ExternalInput").ap()
    d_wo = nc.dram_tensor("woimg", [128, 2 * D], BF16,
                          kind="ExternalInput").ap()
    d_cos = nc.dram_tensor("cos2", [128, T], BF16, kind="ExternalInput").ap()
    d_sin = nc.dram_tensor("sin2", [128, T], BF16, kind="ExternalInput").ap()
    d_mask = nc.dram_tensor("trimask", [128, 256], BF16,
                            kind="ExternalInput").ap()
    d_out = nc.dram_tensor("outp", [D, T], BF16, kind="ExternalOutput").ap()
    dbg = bool(int(os.environ.get("KDEBUG", "0")))
    if dbg:
        d_dbg_q0 = nc.dram_tensor("dbg_q0", [128, T], BF16,
                                  kind="ExternalOutput").ap()
        d_dbg_k0 = nc.dram_tensor("dbg_k0", [128, T], BF16,
                                  kind="ExternalOutput").ap()
        d_dbg_va0 = nc.dram_tensor("dbg_va0", [128, 2 * 16 * 65], BF16,
                                   kind="ExternalOutput").ap()
        d_dbg_o0 = nc.dram_tensor("dbg_o0", [128, T], BF16,
                                  kind="ExternalOutput").ap()

    with TileContext(nc) as tc, nc.allow_low_precision(reason="bf16 attn"):
        with ExitStack() as root:
            persist = root.enter_context(tc.tile_pool(name="persist", bufs=1))

            qT = [persist.tile([128, T], BF16, tag=f"q{p}", name=f"qT{p}")
                  for p in range(2)]
            kT = [persist.tile([128, T], BF16, tag=f"k{p}", name=f"kT{p}")
                  for p in range(2)]
            vap = [persist.tile([128, 2 * 16 * 65], BF16, tag=f"va{p}",
                                name=f"vap{p}") for p in range(2)]
            oT = [persist.tile([128, T], BF16, tag=f"o{p}", name=f"oT{p}")
                  for p in range(2)]
            x_sb = [persist.tile([128, T], BF16, tag=f"x{kt}",
                                 name=f"xsb{kt}") for kt in range(NK)]
            w_sb = [persist.tile([128, NK * 128], BF16, tag=f"w{c}",
                                 name=f"wsb{c}") for c in range(6)]
            wo_sb = [persist.tile([128, D], BF16, tag=f"wo{p}",
                                  name=f"wo{p}") for p in range(2)]
            cos2 = persist.tile([128, T], BF16, tag="cos")
            sin2 = persist.tile([128, T], BF16, tag="sin")
            trimask = persist.tile([128, 256], BF16, tag="m")

            ropep = root.enter_context(tc.tile_pool(name="ropep", bufs=1))
            qc = [ropep.tile([128, T], BF16, tag=f"qc{i}", name=f"qc{i}")
                  for i in range(2)]
            qsh = [ropep.tile([128, T], BF16, tag=f"qsh{i}", name=f"qsh{i}")
                   for i in range(2)]
            qco = [ropep.tile([128, T], BF16, tag=f"qco{i}", name=f"qco{i}")
                   for i in range(2)]

            ptbp = root.enter_context(tc.tile_pool(name="ptbp", bufs=3))
            rp = root.enter_context(tc.tile_pool(name="rp", bufs=2))
            fop = root.enter_context(tc.tile_pool(name="fop",
                                      bufs=int(os.environ.get("KFOP", "8"))))

            # [128, hl, kb, 65] views of vap
            vap3 = [vap[p][:].rearrange("a (h k c) -> a h k c", h=2, k=16)
                    for p in range(2)]
            tri3 = trimask[:].rearrange("a (h c) -> a h c", h=2)

            # ---------------- DMA preamble (sync queue, ordered) ----------
            def wslice(c):
                return d_w[:, c * NK * 128:(c + 1) * NK * 128]

            nc.sync.dma_start(out=x_sb[0][:], in_=d_xT[0:128, :])
            nc.sync.dma_start(out=w_sb[WC_Q0][:], in_=wslice(WC_Q0))
            nc.sync.dma_start(out=w_sb[WC_K0][:], in_=wslice(WC_K0))
            for kt in range(1, NK):
                nc.sync.dma_start(out=x_sb[kt][:],
                                  in_=d_xT[kt * 128:(kt + 1) * 128, :])
            nc.sync.dma_start(out=cos2[:], in_=d_cos[:])
            nc.sync.dma_start(out=sin2[:], in_=d_sin[:])
            nc.sync.dma_start(out=w_sb[WC_V0][:], in_=wslice(WC_V0))
            nc.sync.dma_start(out=w_sb[WC_Q1][:], in_=wslice(WC_Q1))
            nc.sync.dma_start(out=w_sb[WC_K1][:], in_=wslice(WC_K1))
            nc.sync.dma_start(out=w_sb[WC_V1][:], in_=wslice(WC_V1))
            nc.sync.dma_start(out=wo_sb[0][:], in_=d_wo[:, 0:D])
            nc.sync.dma_start(out=wo_sb[1][:], in_=d_wo[:, D:2 * D])
            nc.sync.dma_start(out=trimask[:], in_=d_mask[:])

            for p in range(2):
                nc.vector.memset(vap3[p][:, :, :, 64:65], 1.0)

            # ---------------- rope helpers --------------------------------
            def rope_strip_ops(dst, buf, sl, src):
                """4 DVE ops turning src (bf16 or psum f32 [128, len(sl)])
                into roped dst[:, sl]."""
                nc.vector.tensor_tensor(out=qco[buf][:, sl], in0=src,
                                        in1=cos2[:, sl], op=ALU.mult)
                nc.vector.stream_shuffle(qsh[buf][:, sl], src, SWAP_MASK)
                nc.vector.tensor_tensor(out=qsh[buf][:, sl],
                                        in0=qsh[buf][:, sl],
                                        in1=sin2[:, sl], op=ALU.mult)
                nc.vector.tensor_tensor(out=dst[:, sl], in0=qsh[buf][:, sl],
                                        in1=qco[buf][:, sl], op=ALU.add)

            # ---------------- Phase A: x streaming + pair-0 q/k projs -----
            # Warm up the PE pstate ramp while the first DMAs land, then
            # stream q0 and k0 kt-major (one full psum bank per strip:
            # accumulation groups must own a whole bank since start=True
            # zeroes the 2KB "zero region").
            wtmp = ropep.tile([128, 512], BF16, tag="wtmp")
            nc.vector.memset(wtmp[:], 0.0)
            psW = tc.alloc_tile_pool(name="psW", bufs=1, space="PSUM")
            pw = psW.tile([128, 512], F32, tag="pw")
            for i in range(9):
                nc.tensor.matmul(pw[:], wtmp[0:128, 0:128], wtmp[:],
                                 start=True, stop=True)
            psW.release()

            psQ = tc.alloc_tile_pool(name="psQ", bufs=1, space="PSUM")
            psK = tc.alloc_tile_pool(name="psK", bufs=1, space="PSUM",
                                     side="right")
            pq = [psQ.tile([128, 512], F32, tag=f"pq{st}", name=f"pq{st}")
                  for st in range(4)]
            pk = [psK.tile([128, 512], F32, tag=f"pk{st}", name=f"pk{st}")
                  for st in range(4)]

            wq0 = w_sb[WC_Q0]
            wk0 = w_sb[WC_K0]
            for kt in range(NK):
                ks = slice(kt * 128, (kt + 1) * 128)
                for st in range(4):
                    nc.tensor.matmul(
                        pq[st][:], wq0[:, ks],
                        x_sb[kt][:, st * 512:(st + 1) * 512],
                        start=(kt == 0), stop=(kt == NK - 1))
                for st in range(4):
                    nc.tensor.matmul(
                        pk[st][:], wk0[:, ks],
                        x_sb[kt][:, st * 512:(st + 1) * 512],
                        start=(kt == 0), stop=(kt == NK - 1))

            # psum -> bf16 copies for all 8 strips (ACT gets the two
            # critical ones first; DVE the rest), but only the two rope
            # chains needed to start attention run here — the other six are
            # deferred into the background generator.
            nc.scalar.copy(qc[1][:, 0:512], pk[0][:])
            nc.scalar.copy(qc[0][:, 1536:2048], pq[3][:])
            nc.vector.tensor_copy(qc[1][:, 512:1024], pk[1][:])
            nc.scalar.copy(qc[1][:, 1024:1536], pk[2][:])
            nc.scalar.copy(qc[1][:, 1536:2048], pk[3][:])
            nc.vector.tensor_copy(qc[0][:, 1024:1536], pq[2][:])
            nc.vector.tensor_copy(qc[0][:, 512:1024], pq[1][:])
            nc.vector.tensor_copy(qc[0][:, 0:512], pq[0][:])
            rope_strip_ops(kT[0], 1, slice(0, 512), qc[1][:, 0:512])
            rope_strip_ops(qT[0], 0, slice(1536, 2048),
                           qc[0][:, 1536:2048])
            psK.release()
            psQ.release()

            # ---------------- Phase B pools -------------------------------
            psS = tc.alloc_tile_pool(name="psS", bufs=2, space="PSUM",
                                     side="right")
            psA = tc.alloc_tile_pool(name="psA", bufs=1, space="PSUM")
            psBG = tc.alloc_tile_pool(name="psBG", bufs=2, space="PSUM")
            

            def bg_pair1():
                """Deferred pair-0 rope chains + v0 (kb-major), then q1/k1
                proj+rope, then v1; one psum bank per accumulation group.
                Yields between PE chunks."""
                # (tensor, buf, strip) in the order attention will need them
                deferred = [(kT[0], 1, 1), (qT[0], 0, 2), (kT[0], 1, 2),
                            (qT[0], 0, 1), (kT[0], 1, 3), (qT[0], 0, 0)]
                for p in range(2):
                    if p == 1:
                        while deferred:
                            dst, buf, st = deferred.pop(0)
                            rope_strip_ops(dst, buf,
                                           slice(st * 512, (st + 1) * 512),
                                           qc[buf][:, st * 512:
                                                   (st + 1) * 512])
                            yield
                        # pair-1 q/k projections before its v
                        for wc, dstq, buf in ((WC_Q1, qT[1], 0),
                                              (WC_K1, kT[1], 1)):
                            w = w_sb[wc]
                            for st in range(4):
                                pj = psBG.tile([128, 512], F32, tag="bgk",
                                               name=f"bg{wc}_{st}")
                                for kt in range(0, NK, 2):
                                    for k2 in (kt, kt + 1):
                                        nc.tensor.matmul(
                                            pj[:],
                                            w[:, k2 * 128:(k2 + 1) * 128],
                                            x_sb[k2][:,
                                                     st * 512:(st + 1) * 512],
                                            start=(k2 == 0),
                                            stop=(k2 == NK - 1))
                                    yield
                                sl = slice(st * 512, (st + 1) * 512)
                                nc.vector.tensor_copy(qc[buf][:, sl], pj[:])
                                rope_strip_ops(dstq, buf, sl, qc[buf][:, sl])
                                yield
                    wv = w_sb[WC_V0 if p == 0 else WC_V1]
                    for kb in range(16):
                        if p == 0 and kb % 3 == 2 and deferred:
                            dst, buf, st = deferred.pop(0)
                            rope_strip_ops(dst, buf,
                                           slice(st * 512, (st + 1) * 512),
                                           qc[buf][:, st * 512:
                                                   (st + 1) * 512])
                            yield
                        pv = psBG.tile([128, 512], F32, tag="bgk",
                                       name=f"bgv{p}_{kb}")
                        for kt in range(NK):
                            nc.tensor.matmul(
                                pv[:, 0:128],
                                x_sb[kt][:, kb * 128:(kb + 1) * 128],
                                wv[:, kt * 128:(kt + 1) * 128],
                                start=(kt == 0), stop=(kt == NK - 1))
                        yield
                        if kb % 2 == 0:
                            nc.vector.tensor_copy(
                                vap3[p][:, :, kb, 0:64],
                                pv[:, 0:128].rearrange("a (h c) -> a h c",
                                                       h=2))
                        else:
                            nc.scalar.copy(
                                vap3[p][:, :, kb, 0:64],
                                pv[:, 0:128].rearrange("a (h c) -> a h c",
                                                       h=2))
                        yield

            def bg_oproj(si, pool, alt=False, t0=None, tw=512):
                if t0 is None:
                    t0 = si * 512
                for n in range(8):
                    pD = pool.tile([128, 512], F32, tag="pd",
                                   name=f"pD{si}_{n}_{t0}")
                    for p in range(2):
                        nc.tensor.matmul(
                            pD[:, 0:tw], wo_sb[p][:, n * 128:(n + 1) * 128],
                            oT[p][:, t0:t0 + tw],
                            start=(p == 0), stop=(p == 1))
                    fo = fop.tile([128, 512], BF16, tag="fo")
                    if alt and n % 2 == 1:
                        nc.scalar.copy(fo[:, 0:tw], pD[:, 0:tw])
                    else:
                        nc.vector.tensor_copy(fo[:, 0:tw], pD[:, 0:tw])
                    nc.sync.dma_start(
                        out=d_out[n * 128:(n + 1) * 128, t0:t0 + tw],
                        in_=fo[:, 0:tw])
                    yield

            def drain(gen, count=10 ** 9):
                if gen is None:
                    return
                for _ in range(count):
                    try:
                        next(gen)
                    except StopIteration:
                        return

            def attn_strip(si, p, bg=None, bg_per_kb=2, q_lo=None,
                           q_len=512):
                """Attention for q-strip si, pair p; AV lags one block."""
                q0 = 512 * si if q_lo is None else q_lo
                kb_max = (q0 + q_len) // 128
                av = [psA.tile([65, 512], F32, tag=f"av{hl}",
                               name=f"av{si}_{p}_{hl}_{q0}")
                      for hl in range(2)]

                def emit_av(st):
                    kb, ptb, o, L = st
                    for hl in range(2):
                        nc.tensor.matmul(
                            av[hl][:, o:q_len],
                            vap3[p][:, hl, kb, :],
                            ptb[:, 512 * hl:512 * hl + L],
                            start=(kb == 0), stop=(kb == kb_max - 1),
                            skip_group_check=True)

                prev = None
                for kb in range(kb_max):
                    o = max(0, 128 * kb - q0)
                    L = q_len - o
                    sps = psS.tile([128, 1024], F32, tag="sps",
                                   name=f"sps{si}_{p}_{kb}_{q0}")
                    for hl in range(2):
                        hb = 64 * hl
                        nc.tensor.matmul(
                            sps[:, 512 * hl + o:512 * hl + q_len],
                            kT[p][hb:hb + 64, kb * 128:(kb + 1) * 128],
                            qT[p][hb:hb + 64, q0 + o:q0 + q_len],
                            start=True, stop=True)
                    ptb = ptbp.tile([128, 1024], BF16, tag="ptb",
                                    name=f"ptb{si}_{p}_{kb}_{q0}")
                    sps3 = sps[:].rearrange("a (h q) -> a h q", h=2)
                    ptb3 = ptb[:].rearrange("a (h q) -> a h q", h=2)
                    nc.scalar.activation(ptb3[:, :, 0:L], sps3[:, :, o:q_len],
                                         AF.Exp, scale=SCALE)
                    if 128 * (kb + 1) > q0:
                        nc.vector.tensor_tensor(
                            out=ptb3[:, :, 0:128], in0=ptb3[:, :, 0:128],
                            in1=tri3, op=ALU.mult)
                    if prev is not None:
                        emit_av(prev)
                    prev = (kb, ptb, o, L)
                    if bg is not None:
                        drain(bg, count=bg_per_kb)
                emit_av(prev)
                # normalize -> oT strip
                for hl in range(2):
                    r_sb = rp.tile([1, 512], F32, tag=f"r{hl}",
                                   name=f"rsb{si}_{p}_{hl}_{q0}")
                    nc.vector.reciprocal(r_sb[:, 0:q_len], av[hl][64:65, 0:q_len])
                    rb = rp.tile([64, 512], F32, tag=f"rb{hl}",
                                 name=f"rbb{si}_{p}_{hl}_{q0}")
                    nc.gpsimd.partition_broadcast(rb[:, 0:q_len],
                                                  r_sb[:, 0:q_len])
                    nc.vector.tensor_tensor(
                        out=oT[p][64 * hl:64 * hl + 64, q0:q0 + q_len],
                        in0=av[hl][0:64, 0:q_len], in1=rb[:, 0:q_len],
                        op=ALU.mult)

            # ---------------- Phase B: attention --------------------------
            bg1 = bg_pair1()
            drain(bg1, count=6)
            for si in (3, 2, 1, 0):
                attn_strip(si, 0, bg=bg1, bg_per_kb=3 if si < 3 else 2)

            # leftover pair-1 work fills the first p1 strip (its out-proj
            # is not available yet)
            attn_strip(3, 1, bg=bg1, bg_per_kb=2)
            drain(bg1)
            psBG.release()
            psD = tc.alloc_tile_pool(name="psD", bufs=2, space="PSUM")
            obg = bg_oproj(3, psD)
            for si in (2, 1):
                attn_strip(si, 1, bg=obg, bg_per_kb=1)
                drain(obg)
                obg = bg_oproj(si, psD,
                               alt=(si == 1 and bool(int(os.environ.get("KALT", "1")))))
            # last strip in two 256-col halves so its own out-proj overlaps
            attn_strip(0, 1, bg=obg, bg_per_kb=1, q_lo=0, q_len=256)
            drain(obg)
            obg = bg_oproj(0, psD, alt=bool(int(os.environ.get("KALT", "1"))), t0=0, tw=256)
            attn_strip(0, 1, bg=obg, bg_per_kb=2, q_lo=256, q_len=256)
            drain(obg)
            obg = None

            psD.release()
            psA.release()
            psS.release()

            # tail: out-projection of the last (smallest) strip with all
            # banks available
            psD2 = tc.alloc_tile_pool(name="psD2", bufs=4, space="PSUM")
            drain(bg_oproj(0, psD2, alt=True, t0=256, tw=256))
            psD2.release()

            if dbg:
                nc.sync.dma_start(out=d_dbg_q0[:], in_=qT[0][:])
                nc.sync.dma_start(out=d_dbg_k0[:], in_=kT[0][:])
                nc.sync.dma_start(out=d_dbg_va0[:], in_=vap[0][:])
                nc.sync.dma_start(out=d_dbg_o0[:], in_=oT[0][:])

    nc.compile()
    return nc


_NC_CACHE = None


def _get_program():
    global _NC_CACHE
    if _NC_CACHE is None:
        _NC_CACHE = _build_program()
    return _NC_CACHE


def _rope_tables():
    inv_freq = 1.0 / (10000.0 ** (np.arange(0, HD, 2, dtype=np.float32) / HD))
    freqs = np.outer(np.arange(T, dtype=np.float32), inv_freq)  # [T, 32]
    emb = np.concatenate([freqs, freqs], axis=-1)               # [T, 64]
    return np.cos(emb), np.sin(emb)


def _to_bf16(a):
    import ml_dtypes
    return np.asarray(a, dtype=np.float32).astype(ml_dtypes.bfloat16)


def _swizzle_w(wcol):
    """[D, 128] column block -> SBUF image [128, NK*128] with
    img[p, kt*128 + j] = wcol[kt*128 + p, j]."""
    w3 = wcol.reshape(NK, 128, 128)          # [kt, p, j]
    return np.ascontiguousarray(w3.transpose(1, 0, 2).reshape(128, NK * 128))


def _host_prep(x, w_qkv, w_out):
    cos, sin = _rope_tables()
    cosP = np.ascontiguousarray(cos.T[PI, :])                   # [64, T]
    sinP = sin.T[PI, :].copy()
    sinP[0::2, :] *= -1.0                                       # sign baked in
    cos2 = _to_bf16(np.vstack([cosP, cosP]))
    sin2 = _to_bf16(np.vstack([sinP, sinP]))
    tri = np.triu(np.ones((128, 128), dtype=np.float32))        # keep j >= i
    trimask = _to_bf16(np.ascontiguousarray(
        np.concatenate([tri, tri], axis=1)))                    # [128, 256]

    in_maps = []
    for core in range(NCORES):
        b = core // GROUPS
        h0 = (core % GROUPS) * HPC
        xT = _to_bf16(np.ascontiguousarray(x[b].T))             # [D, T]

        def wcolq(kind, pair):                                  # permuted
            cols = []
            for hh in range(2):
                h = h0 + 2 * pair + hh
                wcol = w_qkv[:, kind * D + h * HD:kind * D + (h + 1) * HD]
                cols.append(wcol[:, PI])
            return np.concatenate(cols, axis=1)                 # [D, 128]

        def wcolv(pair):
            cols = []
            for hh in range(2):
                h = h0 + 2 * pair + hh
                cols.append(w_qkv[:, 2 * D + h * HD:2 * D + (h + 1) * HD])
            return np.concatenate(cols, axis=1)

        order = [wcolq(0, 0), wcolq(1, 0), wcolv(0), wcolv(1),
                 wcolq(0, 1), wcolq(1, 1)]
        wimg = np.concatenate([_swizzle_w(c) for c in order], axis=1)
        wimg = np.ascontiguousarray(_to_bf16(wimg))             # [128, 6144]

        wo_rows = w_out[h0 * HD:(h0 + HPC) * HD, :]             # [256, D]
        woimg = np.ascontiguousarray(_to_bf16(
            np.concatenate([wo_rows[0:128, :], wo_rows[128:256, :]],
                           axis=1)))                            # [128, 2D]

        in_maps.append({
            "xT": xT,
            "wimg": wimg,
            "woimg": woimg,
            "cos2": cos2,
            "sin2": sin2,
            "trimask": trimask,
        })
    return in_maps


def kernel(x, w_qkv, w_out):
    x = np.asarray(x, dtype=np.float32)
    w_qkv = np.asarray(w_qkv, dtype=np.float32)
    w_out = np.asarray(w_out, dtype=np.float32)
    nc = _get_program()
    in_maps = _host_prep(x, w_qkv, w_out)
    trace = bool(int(os.environ.get("KBENCH_TRACE", "0")))
    res = run_bass_kernel_spmd(nc, in_maps, list(range(NCORES)), trace=trace)
    if trace and res.exec_time_ns is not None:
        print(f"HW exec time: {res.exec_time_ns} ns")
    out = np.zeros((B, T, D), dtype=np.float32)
    for core in range(NCORES):
        b = core // GROUPS
        out[b] += res.results[core]["outp"].T.astype(np.float32)
    return out
